# revision 11
# baseline (speedup 1.0000x reference)
"""HRNN Trainium2 kernel: 16 encoders (3-layer tanh RNN + FF) -> 4-layer decoder.

Sharding: expert-parallel, 2 encoders per core across 8 cores; decoder
column-sharded (256 of H_DEC / 128 of D_OUT per core) with AllGathers between
decoder layers. Final output assembled on host from per-core [128, T] shards.

Phase 1 (recurrence) keeps all RNN weights resident in SBUF, weight-stationary
(lhsT = 128x128 Wh tile, rhs = state column). The non-recurrent input term
u_l = H_{l-1} @ W_in_l + b_l is batch-computed directly INTO a PSUM bank
(one bank per encoder, ping-ponged across layer windows); the per-timestep
Wh matvecs then accumulate on top of u in PSUM (start=False), so the whole
per-step tail is a single scalar-engine tanh reading PSUM and writing the
bf16 state column. No vector-engine ops on the recurrent critical path.

States are single bf16 (u stays exact fp32 in PSUM; accumulation fp32);
weights are bf16 except tiny W_in0 (fp32, exact).
"""

import sys
import numpy as np

sys.path.insert(0, "/opt/trn_rl_repo")

import ml_dtypes

E = 16
L = 3
D_IN = 32
D = 512
H_FF = 2048
D_ENC = 512
N_DEC = 4
H_DEC = 2048
D_OUT = 1024
T_FULL = 128
N_CORES = 8

E_LOC = E // N_CORES          # 2 encoders per core
DT = D // 128                 # 4 d-tiles
HD_SH = H_DEC // N_CORES      # 256 decoder hidden per core
HD_SHT = HD_SH // 128         # 2 tiles
DO_SH = D_OUT // N_CORES      # 128 output dims per core
NFT = H_FF // 128             # 16 ff tiles
NCAT = (L * D) // 128         # 12 cat tiles
NDK = (E * D_ENC) // 128      # 64 decoder-input k-tiles
NHD = H_DEC // 128            # 16

BF = ml_dtypes.bfloat16


def _tile_kxm(w):
    """[K, M] -> [128, nk*nm*128] with col ((i*nm)+j)*128 : lhsT tile (i,j)."""
    K, M = w.shape
    nk, nm = K // 128, M // 128
    return np.ascontiguousarray(
        w.reshape(nk, 128, nm, 128).transpose(1, 0, 2, 3).reshape(128, nk * nm * 128)
    )


def _bias_cols(b):
    """[M] -> [128, M//128] with col j holding b[j*128:(j+1)*128]."""
    return np.ascontiguousarray(b.reshape(-1, 128).T)


def build_nc(t_steps):
    from concourse import bacc, bass, mybir, tile

    F32 = mybir.dt.float32
    BF16 = mybir.dt.bfloat16
    AF = mybir.ActivationFunctionType
    BYPASS = mybir.AluOpType.bypass
    T = t_steps

    nc = bacc.Bacc(None, num_devices=N_CORES)

    # ---- I/O declarations -------------------------------------------------
    xT = nc.dram_tensor("xT", [D_IN, T], F32, kind="ExternalInput")
    win0 = [nc.dram_tensor(f"win0_{k}", [D_IN, D], F32, kind="ExternalInput")
            for k in range(E_LOC)]
    wh = [nc.dram_tensor(f"wh_{k}", [128, L * DT * DT * 128], BF16, kind="ExternalInput")
          for k in range(E_LOC)]
    win = [nc.dram_tensor(f"win_{k}", [128, (L - 1) * DT * DT * 128], BF16, kind="ExternalInput")
           for k in range(E_LOC)]
    # RNN biases as a single-partition row: bT[0, (l*DT+j)*128 + m] = b_l[j*128+m]
    bT = [nc.dram_tensor(f"bT_{k}", [1, L * D], BF16, kind="ExternalInput")
          for k in range(E_LOC)]
    ones = nc.dram_tensor("ones", [1, T], BF16, kind="ExternalInput")
    wff1 = [nc.dram_tensor(f"wff1_{k}", [128, NCAT * NFT * 128], BF16, kind="ExternalInput")
            for k in range(E_LOC)]
    bff1 = [nc.dram_tensor(f"bff1_{k}", [128, NFT], F32, kind="ExternalInput")
            for k in range(E_LOC)]
    wff2 = [nc.dram_tensor(f"wff2_{k}", [128, NFT * DT * 128], BF16, kind="ExternalInput")
            for k in range(E_LOC)]
    bff2 = [nc.dram_tensor(f"bff2_{k}", [128, DT], F32, kind="ExternalInput")
            for k in range(E_LOC)]
    wd0 = nc.dram_tensor("wd0", [128, NDK * HD_SHT * 128], BF16, kind="ExternalInput")
    bd0 = nc.dram_tensor("bd0", [128, HD_SHT], F32, kind="ExternalInput")
    wdm = [nc.dram_tensor(f"wdm{m}", [128, NHD * HD_SHT * 128], BF16, kind="ExternalInput")
           for m in range(N_DEC - 2)]
    bdm = [nc.dram_tensor(f"bdm{m}", [128, HD_SHT], F32, kind="ExternalInput")
           for m in range(N_DEC - 2)]
    wdo = nc.dram_tensor("wdo", [128, NHD * 128], BF16, kind="ExternalInput")
    bdo = nc.dram_tensor("bdo", [128, 1], F32, kind="ExternalInput")
    y_out = nc.dram_tensor("y_out", [DO_SH, T], F32, kind="ExternalOutput")

    # collective bounce buffers (ag0 split per local encoder so the first
    # AllGather overlaps the second encoder's FF compute)
    ag0_in = [nc.dram_tensor(f"ag0_in{k}", [D_ENC, T], BF16) for k in range(E_LOC)]
    ag0_out = [nc.dram_tensor(f"ag0_out{k}", [N_CORES * D_ENC, T], BF16,
                              addr_space="Shared") for k in range(E_LOC)]
    agz_in = [nc.dram_tensor(f"agz_in{m}", [HD_SH, T], BF16) for m in range(N_DEC - 1)]
    agz_out = [nc.dram_tensor(f"agz_out{m}", [H_DEC, T], BF16, addr_space="Shared")
               for m in range(N_DEC - 1)]

    RG = [list(range(N_CORES))]

    def colw(i, j, nm):
        return (i * nm + j) * 128

    with tile.TileContext(nc, num_cores=N_CORES) as tc:
        with (
            tc.tile_pool(name="persist", bufs=1) as persist,
            tc.tile_pool(name="dec_w", bufs=1) as dec_w,
            tc.tile_pool(name="ps_u", bufs=1, space="PSUM") as ps_u,
            tc.tile_pool(name="ps_big", bufs=4, space="PSUM") as ps_big,
        ):
            # --- persistent small tensors + state buffers
            xT_sb = persist.tile([D_IN, T], F32, name="xT", tag="xT")
            nc.sync.dma_start(xT_sb[:], xT[:])
            ones_sb = persist.tile([1, T], BF16, name="ones", tag="ones")
            nc.sync.dma_start(ones_sb[:], ones[:])
            win0_sb, bT_sb, bff1_sb, bff2_sb, ench_sb = [], [], [], [], []
            hhl = [[None] * L for _ in range(E_LOC)]
            for k in range(E_LOC):
                w0 = persist.tile([D_IN, D], F32, name=f"win0_{k}", tag=f"win0_{k}")
                nc.sync.dma_start(w0[:], win0[k][:])
                win0_sb.append(w0)
                bb = persist.tile([1, L * D], BF16, name=f"bT_{k}", tag=f"bT_{k}")
                nc.sync.dma_start(bb[:], bT[k][:])
                bT_sb.append(bb)
                b1 = persist.tile([128, NFT], F32, name=f"bff1_{k}", tag=f"bff1_{k}")
                nc.sync.dma_start(b1[:], bff1[k][:])
                bff1_sb.append(b1)
                b2 = persist.tile([128, DT], F32, name=f"bff2_{k}", tag=f"bff2_{k}")
                nc.sync.dma_start(b2[:], bff2[k][:])
                bff2_sb.append(b2)
                for l in range(L):
                    hhl[k][l] = persist.tile([128, DT, T], BF16, name=f"hhl_{k}_{l}", tag=f"hhl_{k}_{l}")
                ench_sb.append(persist.tile([128, DT, T], BF16, name=f"enc_{k}", tag=f"enc_{k}"))
            bd0_sb = persist.tile([128, HD_SHT], F32, name="bd0", tag="bd0")
            nc.sync.dma_start(bd0_sb[:], bd0[:])
            bdm_sb = []
            for m in range(N_DEC - 2):
                t_ = persist.tile([128, HD_SHT], F32, name=f"bdm{m}", tag=f"bdm{m}")
                nc.sync.dma_start(t_[:], bdm[m][:])
                bdm_sb.append(t_)
            bdo_sb = persist.tile([128, 1], F32, name="bdo", tag="bdo")
            nc.sync.dma_start(bdo_sb[:], bdo[:])

            # --- decoder mid/out weights: prefetch early (small)
            wdm_sb = []
            for m in range(N_DEC - 2):
                t_ = dec_w.tile([128, NHD * HD_SHT * 128], BF16, name=f"wdm{m}", tag=f"wdm{m}")
                nc.sync.dma_start(t_[:], wdm[m][:])
                wdm_sb.append(t_)
            wdo_sb = dec_w.tile([128, NHD * 128], BF16, name="wdo", tag="wdo")
            nc.sync.dma_start(wdo_sb[:], wdo[:])

            # u PSUM banks: one bank per (encoder, parity); ping-pong across
            # layer windows. u[k][pp] holds [128, DT, T] fp32 = exactly 2KiB.
            upsum = [[ps_u.tile([128, DT, T], F32, name=f"u_{k}_{pp}", tag=f"u_{k}_{pp}")
                      for pp in range(2)] for k in range(E_LOC)]

            def emit_u_precompute(k, l, pp):
                """u_l = H_{l-1} @ W_in_l + b_l  accumulated into upsum[k][pp].

                First MM (start=True) clears the bank's has_written bits; all
                later MMs use start=False so per-element semantics give
                overwrite-then-accumulate. Per-step Wh matvecs then accumulate
                on top during the window.
                """
                u = upsum[k][pp]
                if l == 0:
                    # u0 = x @ W_in0 (fp32, exact)
                    for j in range(DT):
                        nc.tensor.matmul(u[:, j, :], win0_sb[k][:, j * 128:(j + 1) * 128],
                                         xT_sb[:], start=(j == 0), stop=True)
                    # + b0 (rank-1: bT row x ones)
                    for j in range(DT):
                        nc.tensor.matmul(u[:, j, :],
                                         bT_sb[k][:, (l * DT + j) * 128:(l * DT + j + 1) * 128],
                                         ones_sb[:], start=False, stop=True)
                else:
                    # + b_l first (start=True on j==0 clears the bank)
                    for j in range(DT):
                        nc.tensor.matmul(u[:, j, :],
                                         bT_sb[k][:, (l * DT + j) * 128:(l * DT + j + 1) * 128],
                                         ones_sb[:], start=(j == 0), stop=True)
                    # + H_{l-1} @ W_in_l
                    for j in range(DT):
                        for i in range(DT):
                            nc.tensor.matmul(
                                u[:, j, :],
                                win_sb[k][:, colw((l - 1) * DT + i, j, DT):
                                          colw((l - 1) * DT + i, j, DT) + 128],
                                hhl[k][l - 1][:, i, :],
                                start=False, stop=(i == DT - 1))

            # --- FF1 weights (big): prefetch during phase 1; freed before decoder
            with (
                tc.tile_pool(name="ff1e1", bufs=1) as ff1e1,
                tc.tile_pool(name="ff1e0", bufs=1) as ff1e0,
            ):
                wff1_sb = [None, None]
                wff1_sb[1] = ff1e1.tile([128, NCAT * NFT * 128], BF16, name="wff1_1", tag="wff1_1")
                nc.sync.dma_start(wff1_sb[1][:], wff1[1][:])
                wff1_sb[0] = ff1e0.tile([128, NCAT * NFT * 128], BF16, name="wff1_0", tag="wff1_0")
                nc.sync.dma_start(wff1_sb[0][:], wff1[0][:])

                with tc.tile_pool(name="rnn", bufs=1) as rnn:
                    wh_sb, win_sb = [], []
                    for k in range(E_LOC):
                        t_ = rnn.tile([128, L * DT * DT * 128], BF16, name=f"wh_{k}", tag=f"wh_{k}")
                        nc.sync.dma_start(t_[:], wh[k][:])
                        wh_sb.append(t_)
                        t_ = rnn.tile([128, (L - 1) * DT * DT * 128], BF16, tag=f"win_{k}")
                        nc.sync.dma_start(t_[:], win[k][:])
                        win_sb.append(t_)

                    # ---- phase 1: three layer windows ----
                    for k in range(E_LOC):
                        emit_u_precompute(k, 0, 0)
                    for l in range(L):
                        pp = l % 2
                        for t in range(T):
                            for k in range(E_LOC):
                                u = upsum[k][pp]
                                if t > 0:
                                    for j in range(DT):
                                        for i in range(DT):
                                            nc.tensor.matmul(
                                                u[:, j, t:t + 1],
                                                wh_sb[k][:, colw(l * DT + i, j, DT):
                                                         colw(l * DT + i, j, DT) + 128],
                                                hhl[k][l][:, i, t - 1:t],
                                                start=False, stop=(i == DT - 1))
                                nc.scalar.activation(hhl[k][l][:, :, t:t + 1],
                                                     u[:, :, t:t + 1], AF.Tanh)
                            # next window's u: emit bias early, W_in in chunks
                            # as this window's states complete
                            if l + 1 < L:
                                for k in range(E_LOC):
                                    nu = upsum[k][1 - pp]
                                    if t == 0:
                                        for j in range(DT):
                                            nc.tensor.matmul(
                                                nu[:, j, :],
                                                bT_sb[k][:, ((l + 1) * DT + j) * 128:
                                                         ((l + 1) * DT + j + 1) * 128],
                                                ones_sb[:], start=(j == 0), stop=True)
                                    elif t % 32 == 1 and t > 32:
                                        c = t // 32 - 1  # chunk of 32 steps now complete
                                        for j in range(DT):
                                            for i in range(DT):
                                                nc.tensor.matmul(
                                                    nu[:, j, c * 32:(c + 1) * 32],
                                                    win_sb[k][:, colw(l * DT + i, j, DT):
                                                              colw(l * DT + i, j, DT) + 128],
                                                    hhl[k][l][:, i, c * 32:(c + 1) * 32],
                                                    start=False, stop=(i == DT - 1))
                        # remaining chunk (t in [96, 128)) after window ends
                        if l + 1 < L:
                            for k in range(E_LOC):
                                nu = upsum[k][1 - pp]
                                for j in range(DT):
                                    for i in range(DT):
                                        nc.tensor.matmul(
                                            nu[:, j, 3 * 32:T],
                                            win_sb[k][:, colw(l * DT + i, j, DT):
                                                      colw(l * DT + i, j, DT) + 128],
                                            hhl[k][l][:, i, 3 * 32:T],
                                            start=False, stop=(i == DT - 1))

                # ---- FF phase (rnn pool closed; ff2/ffs pool opens above ff pools) ----
                with tc.tile_pool(name="post1", bufs=1) as post1:
                    wff2_sb, ffs_sb = [], []
                    for k in range(E_LOC):
                        t_ = post1.tile([128, NFT * DT * 128], BF16, name=f"wff2_{k}", tag=f"wff2_{k}")
                        nc.sync.dma_start(t_[:], wff2[k][:])
                        wff2_sb.append(t_)
                        ffs_sb.append(post1.tile([128, NFT, T], BF16, name=f"ffs_{k}", tag=f"ffs_{k}"))

                    for k in range(E_LOC):
                        for m in range(NFT):
                            pf = ps_big.tile([128, T], F32, name="psb", tag="psb")
                            idx = 0
                            for l in range(L):
                                for j in range(DT):
                                    nc.tensor.matmul(
                                        pf[:],
                                        wff1_sb[k][:, colw(l * DT + j, m, NFT):
                                                   colw(l * DT + j, m, NFT) + 128],
                                        hhl[k][l][:, j, :],
                                        start=(idx == 0), stop=(idx == NCAT - 1))
                                    idx += 1
                            nc.scalar.activation(ffs_sb[k][:, m, :], pf[:],
                                                 AF.Gelu_apprx_tanh,
                                                 bias=bff1_sb[k][:, m:m + 1])
                        for j in range(DT):
                            pf2 = ps_big.tile([128, T], F32, name="psb", tag="psb")
                            for i in range(NFT):
                                nc.tensor.matmul(
                                    pf2[:],
                                    wff2_sb[k][:, colw(i, j, DT):colw(i, j, DT) + 128],
                                    ffs_sb[k][:, i, :],
                                    start=(i == 0), stop=(i == NFT - 1))
                            nc.scalar.activation(ench_sb[k][:, j, :], pf2[:], AF.Identity,
                                                 bias=bff2_sb[k][:, j:j + 1])
                        nc.sync.dma_start(
                            ag0_in[k][:].rearrange("(j p) t -> p j t", p=128),
                            ench_sb[k][:, :, :])
                        # trigger this encoder's AllGather right away: the
                        # enc-0 gather overlaps enc-1's FF compute
                        nc.gpsimd.collective_compute(
                            "AllGather", BYPASS, replica_groups=RG,
                            ins=[ag0_in[k][:]], outs=[ag0_out[k][:]])

            # ---- decoder (ff pools closed; their space is reused) ----
            NDK_H = NDK // E_LOC  # 32 k-tiles per encoder-gather
            with tc.tile_pool(name="dec_run", bufs=1) as dec_run:
                wd0_sb = dec_run.tile([128, NDK * HD_SHT * 128], BF16, name="wd0", tag="wd0")
                # chunked so decoder matmuls can chase the DMA
                csz = NDK * HD_SHT * 128 // 8
                for ch in range(8):
                    nc.sync.dma_start(wd0_sb[:, ch * csz:(ch + 1) * csz],
                                      wd0[:, ch * csz:(ch + 1) * csz])
                # cat arrives in two halves (one per encoder AllGather), each
                # DMA'd in 4 chunks spread across engine queues so transfers
                # run in parallel and the d0 matmuls chase them
                dmae = [nc.sync, nc.gpsimd, nc.scalar, nc.sync]
                cat_sb = dec_run.tile([128, NDK, T], BF16, name="cat", tag="cat")
                for h in range(E_LOC):
                    for ch in range(4):
                        i0 = ch * (NDK_H // 4)
                        dmae[ch].dma_start(
                            cat_sb[:, h * NDK_H + i0:h * NDK_H + i0 + NDK_H // 4, :],
                            ag0_out[h][i0 * 128:(i0 + NDK_H // 4) * 128, :].rearrange(
                                "(i p) t -> p i t", p=128))

                zloc = dec_run.tile([128, HD_SHT, T], BF16, name="zloc0", tag="zloc0")
                for j2 in range(HD_SHT):
                    pd = ps_big.tile([128, T], F32, name="psb", tag="psb")
                    for i in range(NDK):
                        nc.tensor.matmul(
                            pd[:],
                            wd0_sb[:, colw(i, j2, HD_SHT):colw(i, j2, HD_SHT) + 128],
                            cat_sb[:, i, :],
                            start=(i == 0), stop=(i == NDK - 1))
                    nc.scalar.activation(zloc[:, j2, :], pd[:], AF.Tanh,
                                         bias=bd0_sb[:, j2:j2 + 1])
                    nc.sync.dma_start(
                        agz_in[0][j2 * 128:(j2 + 1) * 128, :], zloc[:, j2, :])
                nc.gpsimd.collective_compute(
                    "AllGather", BYPASS, replica_groups=RG,
                    ins=[agz_in[0][:]], outs=[agz_out[0][:]])

                for m in range(N_DEC - 2):
                    zf = dec_run.tile([128, NHD, T], BF16, name="zf", tag="zf")
                    for ch in range(4):
                        i0 = ch * (NHD // 4)
                        dmae[ch].dma_start(
                            zf[:, i0:i0 + NHD // 4, :],
                            agz_out[m][i0 * 128:(i0 + NHD // 4) * 128, :].rearrange(
                                "(i p) t -> p i t", p=128))
                    zloc2 = dec_run.tile([128, HD_SHT, T], BF16, name=f"zloc{m + 1}", tag=f"zloc{m + 1}")
                    for j2 in range(HD_SHT):
                        pd = ps_big.tile([128, T], F32, name="psb", tag="psb")
                        for i in range(NHD):
                            nc.tensor.matmul(
                                pd[:],
                                wdm_sb[m][:, colw(i, j2, HD_SHT):colw(i, j2, HD_SHT) + 128],
                                zf[:, i, :],
                                start=(i == 0), stop=(i == NHD - 1))
                        nc.scalar.activation(zloc2[:, j2, :], pd[:], AF.Tanh,
                                             bias=bdm_sb[m][:, j2:j2 + 1])
                        nc.sync.dma_start(
                            agz_in[m + 1][j2 * 128:(j2 + 1) * 128, :], zloc2[:, j2, :])
                    nc.gpsimd.collective_compute(
                        "AllGather", BYPASS, replica_groups=RG,
                        ins=[agz_in[m + 1][:]], outs=[agz_out[m + 1][:]])

                zf3 = dec_run.tile([128, NHD, T], BF16, name="zf", tag="zf")
                for ch in range(4):
                    i0 = ch * (NHD // 4)
                    dmae[ch].dma_start(
                        zf3[:, i0:i0 + NHD // 4, :],
                        agz_out[N_DEC - 2][i0 * 128:(i0 + NHD // 4) * 128, :].rearrange(
                            "(i p) t -> p i t", p=128))
                py = ps_big.tile([128, T], F32, name="psb", tag="psb")
                for i in range(NHD):
                    nc.tensor.matmul(py[:], wdo_sb[:, i * 128:(i + 1) * 128],
                                     zf3[:, i, :], start=(i == 0), stop=(i == NHD - 1))
                y_sb = dec_run.tile([DO_SH, T], F32, name="ysb", tag="ysb")
                nc.scalar.activation(y_sb[:], py[:], AF.Identity, bias=bdo_sb[:])
                nc.sync.dma_start(y_out[:], y_sb[:])

    nc.compile()
    return nc


def prep_inputs(inputs, t_steps):
    """Build the 8 per-core input maps from full numpy inputs."""
    T = t_steps
    f32 = lambda a: np.asarray(a, np.float32)
    x = f32(inputs["x"])
    W_in0, Wh0, b0 = f32(inputs["W_in0"]), f32(inputs["Wh0"]), f32(inputs["b0"])
    W_in_rest, Wh_rest, b_rest = (f32(inputs["W_in_rest"]), f32(inputs["Wh_rest"]),
                                  f32(inputs["b_rest"]))
    W_ff1, b_ff1 = f32(inputs["W_ff1"]), f32(inputs["b_ff1"])
    W_ff2, b_ff2 = f32(inputs["W_ff2"]), f32(inputs["b_ff2"])
    W_d0, b_d0 = f32(inputs["W_d0"]), f32(inputs["b_d0"])
    W_dmid, b_dmid = f32(inputs["W_dmid"]), f32(inputs["b_dmid"])
    W_dout, b_dout = f32(inputs["W_dout"]), f32(inputs["b_dout"])

    xT = np.ascontiguousarray(x[0, :T].T)  # [32, T]
    in_maps = []
    for c in range(N_CORES):
        m = {"xT": xT, "ones": np.ones((1, T), BF)}
        for k in range(E_LOC):
            e = E_LOC * c + k
            m[f"win0_{k}"] = np.ascontiguousarray(W_in0[e])
            wh_all = np.concatenate([Wh0[e][None], Wh_rest[e]], 0)  # [3, D, D]
            m[f"wh_{k}"] = _tile_kxm(wh_all.reshape(L * D, D)).astype(BF)
            m[f"win_{k}"] = _tile_kxm(W_in_rest[e].reshape((L - 1) * D, D)).astype(BF)
            b_all = np.concatenate([b0[e][None], b_rest[e]], 0).reshape(1, -1)
            m[f"bT_{k}"] = b_all.astype(BF)
            m[f"wff1_{k}"] = _tile_kxm(W_ff1[e]).astype(BF)
            m[f"bff1_{k}"] = _bias_cols(b_ff1[e])
            m[f"wff2_{k}"] = _tile_kxm(W_ff2[e]).astype(BF)
            m[f"bff2_{k}"] = _bias_cols(b_ff2[e])
        # rows reordered to match the split per-encoder AllGather layout:
        # first all cores' encoder-0 rows, then all cores' encoder-1 rows
        W_d0_r = np.concatenate(
            [W_d0[(E_LOC * cc + k) * D_ENC:(E_LOC * cc + k + 1) * D_ENC]
             for k in range(E_LOC) for cc in range(N_CORES)], axis=0)
        m["wd0"] = _tile_kxm(W_d0_r[:, c * HD_SH:(c + 1) * HD_SH]).astype(BF)
        m["bd0"] = _bias_cols(b_d0[c * HD_SH:(c + 1) * HD_SH])
        for mm in range(N_DEC - 2):
            m[f"wdm{mm}"] = _tile_kxm(W_dmid[mm][:, c * HD_SH:(c + 1) * HD_SH]).astype(BF)
            m[f"bdm{mm}"] = _bias_cols(b_dmid[mm][c * HD_SH:(c + 1) * HD_SH])
        m["wdo"] = _tile_kxm(W_dout[:, c * DO_SH:(c + 1) * DO_SH]).astype(BF)
        m["bdo"] = _bias_cols(b_dout[c * DO_SH:(c + 1) * DO_SH])
        in_maps.append(m)
    return in_maps


def run(inputs, t_steps=T_FULL, trace=False):
    from concourse.bass_utils import run_bass_kernel_spmd

    nc = build_nc(t_steps)
    in_maps = prep_inputs(inputs, t_steps)
    res = run_bass_kernel_spmd(nc, in_maps, list(range(N_CORES)), trace=trace)
    parts = [res.results[c]["y_out"] for c in range(N_CORES)]  # each [128, T]
    y = np.concatenate([np.asarray(p, np.float32).T for p in parts], axis=1)
    return y[None], res


def kernel(**inputs):
    y, _ = run(inputs, T_FULL, trace=False)
    return y
